# revision 28
# baseline (speedup 1.0000x reference)
"""Trainium2 Bass kernel for nn_AttentionNetwork (ragged path attention).

Data-parallel over 8 NeuronCores: 512 paths per core. Paths are sorted by
length (host-side) and packed into variable-width blocks (bp paths x cap
node-slots, bp*cap <= 1024, cap = max length in the block; capacities are
taken as the element-wise max over cores so one SPMD program serves all
8; the last ~128 paths go into <=256-row blocks so the pipeline drains
quickly). All matmuls run in bf16. Padding slots are filled host-side
with a "poison" vector whose MLP score is ~-3e4, so exp underflows to
exactly 0 and no mask machinery is needed on the device. Per block and
~512-column chunk: node-MLP into paired PSUM banks (one fused relu per
bank pair when b1==0) -> score matmuls -> exp row; the exp row is
broadcast across partitions (GpSimd) into slot KC of a [128, KC+1, rows]
tile whose first KC slots get x*e (one DVE multiply); the segment sums
(weighted feature sums + softmax denominator in slot KC) are folded
pairwise while the width stays even (TensorTensor adds beat TensorReduce
on the DVE), and the partially-folded tile is DMA'd out in bf16. The
host finishes the last few columns of each segment sum, normalizes, and
runs the tiny stage-2 path-attention (0.2% of the FLOPs) in numpy,
combining all 8 cores' exp-weighted partial sums in one pass. This keeps
the device PE-dense to the last stage-1 block with no cross-engine
drain chains at the end.
"""

import sys

if "/opt/trn_rl_repo" not in sys.path:
    sys.path.insert(0, "/opt/trn_rl_repo")

from contextlib import ExitStack

import ml_dtypes
import numpy as np

import concourse.bass as bass  # noqa: F401
import concourse.mybir as mybir
import concourse.tile as tile
from concourse import bacc, bass_utils

P, LMAX, D, H = 4096, 64, 512, 512
NCORES = 8
PS = P // NCORES          # paths per core
KC = D // 128             # contraction chunks
HC = H // 128             # hidden tiles
ROWS_TARGET = 1024        # max rows (bp*cap) per block
TAIL_PATHS = 128          # last paths go into small blocks ...
TAIL_ROWS = 256           # ... of <=256 rows for a fast pipeline drain

f32 = mybir.dt.float32
bf16 = mybir.dt.bfloat16
AF = mybir.ActivationFunctionType
ALU = mybir.AluOpType
AX = mybir.AxisListType

LAST_RESULT = None
_PROG_CACHE = {}
_TRACE_KW = {}


def _fold_width(cap):
    """Fold halves while even and > 2; the host sums the remaining columns."""
    l = cap
    while l > 2 and l % 2 == 0:
        l //= 2
    return l


def _make_blocks(len_max):
    """Greedy pack sorted-desc lengths into (bp, cap) blocks.

    cap and bp are kept even so every matmul free dim / path offset is even.
    """
    blocks = []
    i = 0
    while i < PS:
        cap = int(len_max[i])
        if cap % 2:
            cap += 1
        if i >= PS - 32:
            target = 64          # tiny final blocks: fast warmup DMA + drain
        elif i >= PS - TAIL_PATHS:
            target = TAIL_ROWS
        else:
            target = ROWS_TARGET
        bp = min(max(target // cap, 2), PS - i)
        if bp % 2 and bp > 1:
            bp -= 1
        blocks.append((bp, cap))
        i += bp
    return tuple(blocks)


def _build_program(blocks, b1_zero):
    """blocks: tuple of (bp, cap); one block = bp paths x cap node slots."""
    nb = len(blocks)
    rows_list = [bp * cap for bp, cap in blocks]
    tot_rows = sum(rows_list)
    lf_list = [_fold_width(cap) for bp, cap in blocks]
    q_offs = [0] * nb
    acc_q = 0
    for i in range(nb):
        q_offs[i] = acc_q
        acc_q += 128 * (KC + 1) * blocks[i][0] * lf_list[i]

    nc = bacc.Bacc("TRN2", target_bir_lowering=False, debug=False, num_devices=NCORES)

    xb = nc.dram_tensor("xb", [KC * 128 * tot_rows], bf16, kind="ExternalInput")
    w1 = nc.dram_tensor("w1", [KC, 128, H], bf16, kind="ExternalInput")
    w2 = nc.dram_tensor("w2", [128, HC], bf16, kind="ExternalInput")
    b1 = nc.dram_tensor("b1", [128, HC], f32, kind="ExternalInput")
    praw_d = nc.dram_tensor("praw", [acc_q], bf16, kind="ExternalOutput")

    x_offs = [0] * nb
    acc_x = 0
    for i in range(nb):
        x_offs[i] = acc_x
        acc_x += KC * 128 * rows_list[i]

    with ExitStack() as ctx:
        tc = ctx.enter_context(tile.TileContext(nc))
        const = ctx.enter_context(tc.tile_pool(name="const", bufs=1))
        xpool = ctx.enter_context(tc.tile_pool(name="x", bufs=6))
        xapool = ctx.enter_context(tc.tile_pool(name="xa", bufs=4))
        opool = ctx.enter_context(tc.tile_pool(name="o", bufs=4))
        hpool = ctx.enter_context(tc.tile_pool(name="h", bufs=2))
        spool = ctx.enter_context(tc.tile_pool(name="s", bufs=3))
        ph_pool = ctx.enter_context(tc.tile_pool(name="ph", bufs=3, space="PSUM"))
        ps_pool = ctx.enter_context(tc.tile_pool(name="ps", bufs=2, space="PSUM"))

        # per-k weight tiles: the first matmul only waits on the k=0 slice
        t_w1s = []
        for k in range(KC):
            t_w1k = const.tile([128, H], bf16, name=f"t_w1_{k}")
            nc.sync.dma_start(t_w1k[:], w1.ap()[k])
            t_w1s.append(t_w1k)
        t_w2 = const.tile([128, HC], bf16)
        nc.sync.dma_start(t_w2[:], w2.ap())
        t_b1 = const.tile([128, HC], f32)
        nc.sync.dma_start(t_b1[:], b1.ap())
        # ACT table prefetch: force the exp table load before data arrives
        t_warm = const.tile([1, 1], f32)
        nc.scalar.activation(t_warm[:], t_b1[0:1, 0:1], AF.Exp)

        def mlp(dst, rhs_src, cs, n, wtiles, btile, bzero, tag):
            """dst[:, j, cs] = relu(W_j.T @ rhs + b_j) for j in 0..HC-1."""
            for jj in range(HC // 2):
                # full-bank tile so each [:, j2, :] slice is bank-aligned
                ph = ph_pool.tile([128, 2, 512], f32, tag="h", name=f"ph_{tag}_{jj}")
                for j2 in range(2):
                    j = 2 * jj + j2
                    for k in range(KC):
                        nc.tensor.matmul(
                            ph[:, j2, 0:n],
                            wtiles[k][:, 128 * j : 128 * (j + 1)],
                            rhs_src(k),
                            start=(k == 0),
                            stop=(k == KC - 1),
                        )
                if bzero:
                    nc.scalar.activation(
                        dst[:, 2 * jj : 2 * jj + 2, cs], ph[:, :, 0:n], AF.Relu
                    )
                else:
                    for j2 in range(2):
                        j = 2 * jj + j2
                        nc.scalar.activation(
                            dst[:, j, cs], ph[:, j2, 0:n], AF.Relu,
                            bias=btile[:, j : j + 1],
                        )

        # warm up with two small tail blocks; end with small tail blocks so
        # the last fold chains drain right behind the final matmuls.
        if nb >= 3:
            emit_order = [nb - 1, nb - 2] + list(range(nb - 2))
        else:
            emit_order = list(range(nb))

        for ei, b in enumerate(emit_order):
            bp, cap = blocks[b]
            rows = rows_list[b]
            x_off = x_offs[b]
            lf = lf_list[b]

            x_b = xpool.tile([128, KC, rows], bf16, tag="xb", name=f"xb_{b}")
            nc.sync.dma_start(
                x_b[:],
                xb.ap()[x_off : x_off + KC * 128 * rows].rearrange(
                    "(k d r) -> d k r", k=KC, d=128
                ),
            )

            rh = hpool.tile([128, HC, rows], bf16, tag="rh", name=f"rh_{b}")
            erow = spool.tile([1, rows], bf16, tag="erow", name=f"erow_{b}")
            if rows > 512:
                c0 = ((rows // 2) + 1) // 2 * 2   # even split point
                chunks = [(0, c0), (c0, rows - c0)]
            else:
                chunks = [(0, rows)]
            for (coff, csz) in chunks:
                cs = slice(coff, coff + csz)
                mlp(rh, lambda k: x_b[:, k, cs], cs, csz, t_w1s, t_b1, b1_zero,
                    f"b{b}_{coff}")
                ps_s = ps_pool.tile([1, csz], f32, tag="s", name=f"ps_{b}_{coff}")
                for j in range(HC):
                    nc.tensor.matmul(
                        ps_s[:], t_w2[:, j : j + 1], rh[:, j, cs],
                        start=(j == 0), stop=(j == HC - 1),
                    )
                nc.scalar.activation(erow[:, cs], ps_s[:], AF.Exp)

            # xa holds [x*e (KC chunks) | e] so the fold tree computes the
            # weighted sums and the softmax denominator in one pass.
            xa = xapool.tile([128, KC + 1, rows], bf16, tag="xa", name=f"xa_{b}")
            nc.gpsimd.partition_broadcast(xa[:, KC, :], erow[:])
            nc.vector.tensor_mul(
                xa[:, 0:KC, :],
                x_b[:],
                xa[:, KC, :]
                .rearrange("p (x r) -> p x r", x=1)
                .to_broadcast([128, KC, rows]),
            )

            # fold into a compact exactly-sized tile at the last level so the
            # praw DMA is one contiguous run per partition (128 descriptors)
            halvings = []
            l = cap
            while l > 2 and l % 2 == 0:
                l //= 2
                halvings.append(l)
            assert l == lf

            cur = xa[:].rearrange("p c (s l) -> p c s l", l=cap)
            out_t = opool.tile(
                [128, KC + 1, bp * lf], bf16, tag="praw", name=f"praw_{b}"
            )
            if halvings:
                fs = xapool.tile(
                    [128, KC + 1, rows // 2], bf16, tag="fold", name=f"fold_{b}"
                )
                toggle = 0
                lc = cap
                for li, half in enumerate(halvings):
                    last = li == len(halvings) - 1
                    if last:
                        dst = out_t[:].rearrange("p c (s l) -> p c s l", l=half)
                    else:
                        dst_tile = fs if toggle == 0 else xa
                        dst = dst_tile[:, :, 0 : bp * half].rearrange(
                            "p c (s l) -> p c s l", l=half
                        )
                    nc.vector.tensor_add(
                        dst, cur[:, :, :, 0:half], cur[:, :, :, half:lc]
                    )
                    cur = dst
                    lc = half
                    toggle ^= 1
                src = out_t[:]
            else:
                src = xa[:].rearrange("p c r -> p (c r)")  # rare: cap == 2
                out_t = None

            nc.sync.dma_start(
                praw_d.ap()[
                    q_offs[b] : q_offs[b] + 128 * (KC + 1) * bp * lf
                ].rearrange("(p cs) -> p cs", p=128),
                src.rearrange("p c s -> p (c s)") if out_t is not None else src,
            )
    nc.compile()
    return nc


def _get_program(blocks, b1_zero):
    key = (blocks, b1_zero)
    if key not in _PROG_CACHE:
        _PROG_CACHE[key] = _build_program(blocks, b1_zero)
    return _PROG_CACHE[key]


def _find_poison(pW1, pb1, pw2):
    """x* (bf16) whose node-MLP score is <= -2e4: exp(score) == 0 exactly."""
    rng = np.random.default_rng(12345)
    v = rng.standard_normal((64, D)).astype(np.float32)
    s_inf = np.maximum(v @ pW1, 0.0) @ pw2        # score slope along t*v
    i = int(np.argmin(s_inf))
    s = float(s_inf[i])
    if s > -0.05:
        s = -0.05
        v[i] *= 0.0
        v[i, 0] = 1.0  # degenerate fallback; never hit for random weights
    t = 30000.0 / (-s)
    xpad = (t * v[i]).astype(ml_dtypes.bfloat16)
    # verify with the exact bf16 values (f32 arithmetic, bias included)
    sc = float(
        np.maximum(xpad.astype(np.float32) @ pW1 + pb1, 0.0) @ pw2
    )
    if sc > -2e4:  # extremely unlikely; rescale using measured slope
        xpad = (xpad.astype(np.float32) * (3e4 / max(-sc, 1.0))).astype(
            ml_dtypes.bfloat16
        )
    return xpad


def _prep(inputs):
    """Host-side sharding/sorting/packing."""
    x = np.asarray(inputs["paths_nodes"], dtype=np.float32)
    lengths = np.asarray(inputs["lengths"], dtype=np.int32)
    pW1 = np.asarray(inputs["pW1"], dtype=np.float32)
    pb1 = np.asarray(inputs["pb1"], dtype=np.float32)
    pw2 = np.asarray(inputs["pw2"], dtype=np.float32)
    b1_zero = bool(np.all(pb1 == 0.0))

    bf = ml_dtypes.bfloat16
    len_sh = lengths.reshape(NCORES, PS)
    orders = np.argsort(-len_sh, axis=1, kind="stable")        # [NC, PS] desc
    sorted_len = np.take_along_axis(len_sh, orders, axis=1)
    len_max = sorted_len.max(axis=0)                           # [PS]
    blocks = _make_blocks(len_max)

    xpad = _find_poison(pW1, pb1, pw2).astype(np.float32)

    x_sh = x.reshape(NCORES, PS, LMAX, D)
    w1_np = np.ascontiguousarray(pW1.reshape(KC, 128, H)).astype(bf)
    w2_np = np.ascontiguousarray(pw2.reshape(HC, 128).T).astype(bf)
    b1_np = np.ascontiguousarray(pb1.reshape(HC, 128).T).astype(np.float32)

    ar = np.arange(LMAX + 2)
    in_maps = []
    for c in range(NCORES):
        xc = x_sh[c][orders[c]]                       # [PS, LMAX, D] sorted
        lc = sorted_len[c]                            # [PS]
        xr_parts = []
        p = 0
        for (bp, cap) in blocks:
            lb = lc[p : p + bp]
            if cap <= LMAX:
                xblk = xc[p : p + bp, :cap, :].copy() # [bp, cap, D]
            else:
                xblk = np.concatenate(
                    [
                        xc[p : p + bp, :, :],
                        np.zeros((bp, cap - LMAX, D), dtype=np.float32),
                    ],
                    axis=1,
                )
            pad = ar[None, :cap] >= lb[:, None]       # [bp, cap]
            xblk[pad] = xpad[None, :]
            xb_t = (
                xblk.reshape(bp, cap, KC, 128)
                .transpose(2, 3, 0, 1)
                .reshape(KC, 128, bp * cap)
            )
            xr_parts.append(xb_t.astype(bf).ravel())
            p += bp
        in_maps.append(
            {
                "xb": np.concatenate(xr_parts),
                "w1": w1_np,
                "w2": w2_np,
                "b1": b1_np,
            }
        )
    return blocks, b1_zero, in_maps


def kernel(**inputs):
    global LAST_RESULT
    blocks, b1_zero, in_maps = _prep(inputs)
    nc = _get_program(blocks, b1_zero)

    res = bass_utils.run_bass_kernel_spmd(
        nc, in_maps, core_ids=list(range(NCORES)), **_TRACE_KW
    )
    LAST_RESULT = res

    aW1 = np.asarray(inputs["aW1"], dtype=np.float32)
    ab1 = np.asarray(inputs["ab1"], dtype=np.float32)
    aw2 = np.asarray(inputs["aw2"], dtype=np.float32)

    # host: finish segment sums, normalize, stage-2 path attention (tiny)
    nb = len(blocks)
    lf_list = [_fold_width(cap) for bp, cap in blocks]
    pf_all = []
    for c in range(NCORES):
        praw = np.asarray(res.results[c]["praw"], dtype=np.float32)
        q = 0
        pf_core = np.empty((PS, D), dtype=np.float32)
        p = 0
        for i, (bp, cap) in enumerate(blocks):
            lf = lf_list[i]
            seg = praw[q : q + 128 * (KC + 1) * bp * lf].reshape(
                128, KC + 1, bp, lf
            ).sum(axis=3)
            q += 128 * (KC + 1) * bp * lf
            pf = seg[:, 0:KC, :] / seg[:, KC : KC + 1, :]     # [128, KC, bp]
            pf_core[p : p + bp] = pf.transpose(2, 1, 0).reshape(bp, D)
            p += bp
        pf_all.append(pf_core)
    pf_all = np.concatenate(pf_all, axis=0)                   # [P, D] (sorted)

    h2 = np.maximum(pf_all @ aW1 + ab1, 0.0)
    a = h2 @ aw2                                              # [P]
    a -= a.max()
    ea = np.exp(a)
    user = (ea @ pf_all) / ea.sum()
    return user.astype(np.float32)


# revision 30
# speedup vs baseline: 1.0476x; 1.0476x over previous
"""Trainium2 Bass kernel for nn_AttentionNetwork (ragged path attention).

Data-parallel over 8 NeuronCores: 512 paths per core. Paths are sorted by
length (host-side) and packed into variable-width blocks (bp paths x cap
node-slots, bp*cap <= 1024, cap = max length in the block; capacities are
taken as the element-wise max over cores so one SPMD program serves all
8; the last ~128 paths go into <=256-row blocks so the pipeline drains
quickly). All matmuls run in bf16. Padding slots are filled host-side
with a "poison" vector whose MLP score is ~-3e4, so exp underflows to
exactly 0 and no mask machinery is needed on the device. Per block and
~512-column chunk: node-MLP into paired PSUM banks (one fused relu per
bank pair when b1==0) -> score matmuls -> exp row; the exp row is
broadcast across partitions (GpSimd) into slot KC of a [128, KC+1, rows]
tile whose first KC slots get x*e (one DVE multiply); the segment sums
(weighted feature sums + softmax denominator in slot KC) are folded
pairwise while the width stays even (TensorTensor adds beat TensorReduce
on the DVE), and the partially-folded tile is DMA'd out in bf16. The
host finishes the last few columns of each segment sum, normalizes, and
runs the tiny stage-2 path-attention (0.2% of the FLOPs) in numpy,
combining all 8 cores' exp-weighted partial sums in one pass. This keeps
the device PE-dense to the last stage-1 block with no cross-engine
drain chains at the end.
"""

import sys

if "/opt/trn_rl_repo" not in sys.path:
    sys.path.insert(0, "/opt/trn_rl_repo")

from contextlib import ExitStack

import ml_dtypes
import numpy as np

import concourse.bass as bass  # noqa: F401
import concourse.mybir as mybir
import concourse.tile as tile
from concourse import bacc, bass_utils

P, LMAX, D, H = 4096, 64, 512, 512
NCORES = 8
PS = P // NCORES          # paths per core
KC = D // 128             # contraction chunks
HC = H // 128             # hidden tiles
ROWS_TARGET = 1024        # max rows (bp*cap) per block
TAIL_PATHS = 128          # last paths go into small blocks ...
TAIL_ROWS = 256           # ... of <=256 rows for a fast pipeline drain

f32 = mybir.dt.float32
bf16 = mybir.dt.bfloat16
AF = mybir.ActivationFunctionType
ALU = mybir.AluOpType
AX = mybir.AxisListType

LAST_RESULT = None
_PROG_CACHE = {}
_TRACE_KW = {}


def _fold_width(cap):
    """Fold halves while even and > 2; the host sums the remaining columns."""
    l = cap
    while l > 2 and l % 2 == 0:
        l //= 2
    return l


def _make_blocks(len_max):
    """Greedy pack sorted-desc lengths into (bp, cap) blocks.

    cap and bp are kept even so every matmul free dim / path offset is even.
    """
    blocks = []
    i = 0
    while i < PS:
        cap = int(len_max[i])
        if cap % 2:
            cap += 1
        if i >= PS - 32:
            target = 64          # tiny final blocks: fast warmup DMA + drain
        elif i >= PS - TAIL_PATHS:
            target = TAIL_ROWS
        else:
            target = ROWS_TARGET
        bp = min(max(target // cap, 2), PS - i)
        if bp % 2 and bp > 1:
            bp -= 1
        blocks.append((bp, cap))
        i += bp
    return tuple(blocks)


def _build_program(blocks, b1_zero):
    """blocks: tuple of (bp, cap); one block = bp paths x cap node slots."""
    nb = len(blocks)
    rows_list = [bp * cap for bp, cap in blocks]
    tot_rows = sum(rows_list)
    lf_list = [_fold_width(cap) for bp, cap in blocks]
    q_offs = [0] * nb
    acc_q = 0
    for i in range(nb):
        q_offs[i] = acc_q
        acc_q += 128 * (KC + 1) * blocks[i][0] * lf_list[i]

    nc = bacc.Bacc("TRN2", target_bir_lowering=False, debug=False, num_devices=NCORES)

    xb = nc.dram_tensor("xb", [KC * 128 * tot_rows], bf16, kind="ExternalInput")
    w1 = nc.dram_tensor("w1", [KC, 128, H], bf16, kind="ExternalInput")
    w2 = nc.dram_tensor("w2", [128, HC], bf16, kind="ExternalInput")
    b1 = nc.dram_tensor("b1", [128, HC], f32, kind="ExternalInput")
    praw_d = nc.dram_tensor("praw", [acc_q], bf16, kind="ExternalOutput")

    x_offs = [0] * nb
    acc_x = 0
    for i in range(nb):
        x_offs[i] = acc_x
        acc_x += KC * 128 * rows_list[i]

    with ExitStack() as ctx:
        tc = ctx.enter_context(tile.TileContext(nc))
        const = ctx.enter_context(tc.tile_pool(name="const", bufs=1))
        xpool = ctx.enter_context(tc.tile_pool(name="x", bufs=6))
        xapool = ctx.enter_context(tc.tile_pool(name="xa", bufs=4))
        opool = ctx.enter_context(tc.tile_pool(name="o", bufs=4))
        hpool = ctx.enter_context(tc.tile_pool(name="h", bufs=2))
        spool = ctx.enter_context(tc.tile_pool(name="s", bufs=3))
        ph_pool = ctx.enter_context(tc.tile_pool(name="ph", bufs=3, space="PSUM"))
        ps_pool = ctx.enter_context(tc.tile_pool(name="ps", bufs=2, space="PSUM"))

        # per-k weight tiles: the first matmul only waits on the k=0 slice
        t_w1s = []
        for k in range(KC):
            t_w1k = const.tile([128, H], bf16, name=f"t_w1_{k}")
            nc.sync.dma_start(t_w1k[:], w1.ap()[k])
            t_w1s.append(t_w1k)
        t_w2 = const.tile([128, HC], bf16)
        nc.sync.dma_start(t_w2[:], w2.ap())
        t_b1 = const.tile([128, HC], f32)
        nc.sync.dma_start(t_b1[:], b1.ap())
        # ACT table prefetch: force the exp table load before data arrives
        t_warm = const.tile([1, 1], f32)
        nc.scalar.activation(t_warm[:], t_b1[0:1, 0:1], AF.Exp)

        def mlp(dst, rhs_src, cs, n, wtiles, btile, bzero, tag):
            """dst[:, j, cs] = relu(W_j.T @ rhs + b_j) for j in 0..HC-1."""
            for jj in range(HC // 2):
                # full-bank tile so each [:, j2, :] slice is bank-aligned
                ph = ph_pool.tile([128, 2, 512], f32, tag="h", name=f"ph_{tag}_{jj}")
                for j2 in range(2):
                    j = 2 * jj + j2
                    for k in range(KC):
                        nc.tensor.matmul(
                            ph[:, j2, 0:n],
                            wtiles[k][:, 128 * j : 128 * (j + 1)],
                            rhs_src(k),
                            start=(k == 0),
                            stop=(k == KC - 1),
                        )
                if bzero:
                    nc.scalar.activation(
                        dst[:, 2 * jj : 2 * jj + 2, cs], ph[:, :, 0:n], AF.Relu
                    )
                else:
                    for j2 in range(2):
                        j = 2 * jj + j2
                        nc.scalar.activation(
                            dst[:, j, cs], ph[:, j2, 0:n], AF.Relu,
                            bias=btile[:, j : j + 1],
                        )

        # warm up with two small tail blocks; end with small tail blocks so
        # the last fold chains drain right behind the final matmuls.
        if nb >= 3:
            emit_order = [nb - 1, nb - 2] + list(range(nb - 2))
        else:
            emit_order = list(range(nb))

        for ei, b in enumerate(emit_order):
            bp, cap = blocks[b]
            rows = rows_list[b]
            x_off = x_offs[b]
            lf = lf_list[b]

            x_b = xpool.tile([128, KC, rows], bf16, tag="xb", name=f"xb_{b}")
            nc.sync.dma_start(
                x_b[:],
                xb.ap()[x_off : x_off + KC * 128 * rows].rearrange(
                    "(k d r) -> d k r", k=KC, d=128
                ),
            )

            rh = hpool.tile([128, HC, rows], bf16, tag="rh", name=f"rh_{b}")
            erow = spool.tile([1, rows], bf16, tag="erow", name=f"erow_{b}")
            if rows > 512:
                c0 = ((rows // 2) + 1) // 2 * 2   # even split point
                chunks = [(0, c0), (c0, rows - c0)]
            else:
                chunks = [(0, rows)]
            for (coff, csz) in chunks:
                cs = slice(coff, coff + csz)
                mlp(rh, lambda k: x_b[:, k, cs], cs, csz, t_w1s, t_b1, b1_zero,
                    f"b{b}_{coff}")
                ps_s = ps_pool.tile([1, csz], f32, tag="s", name=f"ps_{b}_{coff}")
                for j in range(HC):
                    nc.tensor.matmul(
                        ps_s[:], t_w2[:, j : j + 1], rh[:, j, cs],
                        start=(j == 0), stop=(j == HC - 1),
                    )
                nc.scalar.activation(erow[:, cs], ps_s[:], AF.Exp)

            # xa holds [x*e (KC chunks) | e] so the fold tree computes the
            # weighted sums and the softmax denominator in one pass.
            xa = xapool.tile([128, KC + 1, rows], bf16, tag="xa", name=f"xa_{b}")
            nc.gpsimd.partition_broadcast(xa[:, KC, :], erow[:])
            nc.vector.tensor_mul(
                xa[:, 0:KC, :],
                x_b[:],
                xa[:, KC, :]
                .rearrange("p (x r) -> p x r", x=1)
                .to_broadcast([128, KC, rows]),
            )

            # fold into a compact exactly-sized tile at the last level so the
            # praw DMA is one contiguous run per partition (128 descriptors)
            halvings = []
            l = cap
            while l > 2 and l % 2 == 0:
                l //= 2
                halvings.append(l)
            assert l == lf

            cur = xa[:].rearrange("p c (s l) -> p c s l", l=cap)
            out_t = opool.tile(
                [128, KC + 1, bp * lf], bf16, tag="praw", name=f"praw_{b}"
            )
            if halvings:
                fs = xapool.tile(
                    [128, KC + 1, rows // 2], bf16, tag="fold", name=f"fold_{b}"
                )
                toggle = 0
                lc = cap
                for li, half in enumerate(halvings):
                    last = li == len(halvings) - 1
                    if last:
                        dst = out_t[:].rearrange("p c (s l) -> p c s l", l=half)
                    else:
                        dst_tile = fs if toggle == 0 else xa
                        dst = dst_tile[:, :, 0 : bp * half].rearrange(
                            "p c (s l) -> p c s l", l=half
                        )
                    nc.vector.tensor_add(
                        dst, cur[:, :, :, 0:half], cur[:, :, :, half:lc]
                    )
                    cur = dst
                    lc = half
                    toggle ^= 1
                src = out_t[:]
            else:
                src = xa[:].rearrange("p c r -> p (c r)")  # rare: cap == 2
                out_t = None

            nc.sync.dma_start(
                praw_d.ap()[
                    q_offs[b] : q_offs[b] + 128 * (KC + 1) * bp * lf
                ].rearrange("(p cs) -> p cs", p=128),
                src.rearrange("p c s -> p (c s)") if out_t is not None else src,
            )
    nc.compile()
    return nc


def _get_program(blocks, b1_zero):
    key = (blocks, b1_zero)
    if key not in _PROG_CACHE:
        _PROG_CACHE[key] = _build_program(blocks, b1_zero)
    return _PROG_CACHE[key]


def _find_poison(pW1, pb1, pw2):
    """x* (bf16) whose node-MLP score is <= -2e4: exp(score) == 0 exactly."""
    rng = np.random.default_rng(12345)
    v = rng.standard_normal((64, D)).astype(np.float32)
    s_inf = np.maximum(v @ pW1, 0.0) @ pw2        # score slope along t*v
    i = int(np.argmin(s_inf))
    s = float(s_inf[i])
    if s > -0.05:
        s = -0.05
        v[i] *= 0.0
        v[i, 0] = 1.0  # degenerate fallback; never hit for random weights
    t = 30000.0 / (-s)
    xpad = (t * v[i]).astype(ml_dtypes.bfloat16)
    # verify with the exact bf16 values (f32 arithmetic, bias included)
    sc = float(
        np.maximum(xpad.astype(np.float32) @ pW1 + pb1, 0.0) @ pw2
    )
    if sc > -2e4:  # extremely unlikely; rescale using measured slope
        xpad = (xpad.astype(np.float32) * (3e4 / max(-sc, 1.0))).astype(
            ml_dtypes.bfloat16
        )
    return xpad


def _prep(inputs):
    """Host-side sharding/sorting/packing."""
    x = np.asarray(inputs["paths_nodes"], dtype=np.float32)
    lengths = np.asarray(inputs["lengths"], dtype=np.int32)
    pW1 = np.asarray(inputs["pW1"], dtype=np.float32)
    pb1 = np.asarray(inputs["pb1"], dtype=np.float32)
    pw2 = np.asarray(inputs["pw2"], dtype=np.float32)
    b1_zero = bool(np.all(pb1 == 0.0))

    bf = ml_dtypes.bfloat16
    # Global sort by length, dealt round-robin: every core's sorted length
    # profile is nearly identical, so the SPMD cross-core cap padding ~0.
    glob = np.argsort(-lengths, kind="stable")                 # [P] desc
    core_paths = glob.reshape(PS, NCORES).T                    # [NC, PS] sorted
    sorted_len = lengths[core_paths]                           # [NC, PS] desc
    len_max = sorted_len.max(axis=0)                           # [PS]
    blocks = _make_blocks(len_max)

    xpad = _find_poison(pW1, pb1, pw2).astype(np.float32)

    w1_np = np.ascontiguousarray(pW1.reshape(KC, 128, H)).astype(bf)
    w2_np = np.ascontiguousarray(pw2.reshape(HC, 128).T).astype(bf)
    b1_np = np.ascontiguousarray(pb1.reshape(HC, 128).T).astype(np.float32)

    ar = np.arange(LMAX + 2)
    in_maps = []
    for c in range(NCORES):
        xc = x[core_paths[c]]                         # [PS, LMAX, D] sorted
        lc = sorted_len[c]                            # [PS]
        xr_parts = []
        p = 0
        for (bp, cap) in blocks:
            lb = lc[p : p + bp]
            if cap <= LMAX:
                xblk = xc[p : p + bp, :cap, :].copy() # [bp, cap, D]
            else:
                xblk = np.concatenate(
                    [
                        xc[p : p + bp, :, :],
                        np.zeros((bp, cap - LMAX, D), dtype=np.float32),
                    ],
                    axis=1,
                )
            pad = ar[None, :cap] >= lb[:, None]       # [bp, cap]
            xblk[pad] = xpad[None, :]
            xb_t = (
                xblk.reshape(bp, cap, KC, 128)
                .transpose(2, 3, 0, 1)
                .reshape(KC, 128, bp * cap)
            )
            xr_parts.append(xb_t.astype(bf).ravel())
            p += bp
        in_maps.append(
            {
                "xb": np.concatenate(xr_parts),
                "w1": w1_np,
                "w2": w2_np,
                "b1": b1_np,
            }
        )
    return blocks, b1_zero, in_maps


def kernel(**inputs):
    global LAST_RESULT
    blocks, b1_zero, in_maps = _prep(inputs)
    nc = _get_program(blocks, b1_zero)

    res = bass_utils.run_bass_kernel_spmd(
        nc, in_maps, core_ids=list(range(NCORES)), **_TRACE_KW
    )
    LAST_RESULT = res

    aW1 = np.asarray(inputs["aW1"], dtype=np.float32)
    ab1 = np.asarray(inputs["ab1"], dtype=np.float32)
    aw2 = np.asarray(inputs["aw2"], dtype=np.float32)

    # host: finish segment sums, normalize, stage-2 path attention (tiny)
    nb = len(blocks)
    lf_list = [_fold_width(cap) for bp, cap in blocks]
    pf_all = []
    for c in range(NCORES):
        praw = np.asarray(res.results[c]["praw"], dtype=np.float32)
        q = 0
        pf_core = np.empty((PS, D), dtype=np.float32)
        p = 0
        for i, (bp, cap) in enumerate(blocks):
            lf = lf_list[i]
            seg = praw[q : q + 128 * (KC + 1) * bp * lf].reshape(
                128, KC + 1, bp, lf
            ).sum(axis=3)
            q += 128 * (KC + 1) * bp * lf
            pf = seg[:, 0:KC, :] / seg[:, KC : KC + 1, :]     # [128, KC, bp]
            pf_core[p : p + bp] = pf.transpose(2, 1, 0).reshape(bp, D)
            p += bp
        pf_all.append(pf_core)
    pf_all = np.concatenate(pf_all, axis=0)                   # [P, D] (sorted)

    h2 = np.maximum(pf_all @ aW1 + ab1, 0.0)
    a = h2 @ aw2                                              # [P]
    a -= a.max()
    ea = np.exp(a)
    user = (ea @ pf_all) / ea.sum()
    return user.astype(np.float32)
